# revision 2
# baseline (speedup 1.0000x reference)
# Min-plus (tropical) matmul kernel for Trainium2, 8 NeuronCores.
#
#   y[n,o] = min_i (x[n,i] + w[o,i]) + bias[o]
#
# Softmin via ordinary matmuls with NO per-row centering:
#
#   ET[i,n] = exp(-t x[n,i] + RBx)
#   Ew[o,i] = exp(-t (w2[o,i] - b_o) + RBw)     w2 = w + bias, b_o = min_i w2[o,i]
#   ST[o,n] = sum_i EwT[i,o] ET[i,n]            (bf16 matmul, fp32 psum)
#   y[n,o]  = -(1/t) ln ST[o,n] + b_o + (RBx+RBw)/t + CSHIFT
#
# t, RBx, RBw are chosen on the host from the exact max|x| and the max
# weight-row range so every exp factor is a NORMAL bf16 number and the
# fp32 PSUM sum cannot overflow; no centering subtract / rank-1 add-back
# is needed.  CSHIFT (the systematic softmin-vs-min gap) is calibrated on
# a small host sample.  The matmul is emitted OUTPUT-TRANSPOSED (o on
# partitions) so the bias/scale fixup is a per-partition tensor_scalar
# (DVE 2x mode); the host untransposes the returned shard.
#
# Per-core device pipeline (4096 rows = 32 row-blocks, chunked):
#   DMA in : x fp16 (p blk) layout, w-side constants precomputed on host
#   PE     : XT_b = X_b^T via matmul-with-identity (fp16 in, fp32 psum)
#            ST_b = Ewt^T @ ET_b (bf16)
#   ACT    : ET = Exp(-t XT + RBx)  psum -> sbuf bf16
#            L  = Ln(ST)            psum -> sbuf fp32
#   DVE    : Y  = (-1/t) L + bb_col (tensor_scalar, 2x_2p) -> fp16
#   DMA out: y^T fp16

import numpy as np
from contextlib import ExitStack

import concourse.bass as bass
import concourse.mybir as mybir
import concourse.tile as tile
from concourse import bacc
from concourse import bass_utils
from concourse.masks import make_identity

FP = mybir.dt.float32
BF16 = mybir.dt.bfloat16
F16 = mybir.dt.float16
AF = mybir.ActivationFunctionType
OP = mybir.AluOpType

N_CORES = 8
DIN = 128
DOUT = 128
GBMAX = 8        # psum ring tile size (blocks)
WARM = 16        # PE warmup matmuls


LN_DOM = 43.5     # |ln s| domain the ACT Ln table covers accurately (~±45)
EXP_LO = -86.0    # exp factors kept normal bf16: arg in [-86, 87]
EXP_HI = 87.0
LN128 = 4.86


def softmin_cfg(x_max: float, rw: float, amin: float, qmax: float):
    """t and the raises RBx/RBw such that (a) every ln(S) lands inside the
    ACT Ln table's accurate domain |ln S| <= LN_DOM, (b) every exp factor is
    a normal bf16 value inside the Exp table domain.

    ln S = RBx + RBw - t*q + ln(sum of relative terms), q = min_i(x+w2) - b_o
    with q in [amin, qmax] (host-computed exact bounds), sum-term in [0, ln128].
    """
    rw = max(rw, 1e-3)
    x_max = max(x_max, 1e-3)
    spread = max(qmax - amin, 1e-3)
    t = min((2.0 * LN_DOM - LN128 - 1.0) / spread, 4000.0)
    rb2 = LN_DOM - LN128 + t * amin          # centers ln S in [-LN_DOM, LN_DOM]
    # split rb2 = rbx + rbw inside the bf16-normality windows
    lo = max(t * rw + EXP_LO, rb2 - EXP_HI + t * x_max)
    hi = min(EXP_HI, rb2 - EXP_LO - t * x_max)
    assert lo <= hi + 1e-6, (t, rb2, lo, hi)
    rbw = 0.5 * (lo + hi)
    rbx = rb2 - rbw
    return t, rbx, rbw


def make_chunks(blk: int) -> list[int]:
    # small chunks at the edges (short fill/drain), 8-block chunks inside
    assert blk >= 12 and blk % 2 == 0
    out, rem = [4, 6], blk - 10
    while rem > 6:
        out.append(min(8, rem - 6))
        rem -= out[-1]
    out.extend([rem - 2, 2])
    return out


def make_loads(chunks: list[int]) -> list:
    # x chunks in order with the small ewt/bb loads interleaved after the
    # second x load (early enough for mm0 / fuse0, without delaying x1)
    return list(chunks[:2]) + ["E", "B"] + list(chunks[2:])


def minplus_body(tc, outs, ins, cfg):
    nc = tc.nc
    t = cfg["t"]
    SH = cfg["shard_rows"]
    BLK = SH // 128
    CS = cfg["chunks"]
    ST_ = [sum(CS[:j]) for j in range(len(CS))]
    NG = len(CS)
    assert sum(CS) == BLK and max(CS) <= GBMAX

    # row r of the shard lives at (partition r // BLK, block r % BLK): per
    # partition each DMA moves cb*256B contiguous bytes.
    xd = ins["x"].rearrange("(p blk) i -> p blk i", p=128)
    # y is stored transposed: y_t[o, blk, p] = y[p*BLK + blk, o]
    yd = outs["y"].rearrange("o (blk n) -> o blk n", n=128)

    with ExitStack() as ctx:
        big = ctx.enter_context(tc.tile_pool(name="big", bufs=1))
        psum = ctx.enter_context(tc.tile_pool(name="psum", bufs=2, space="PSUM"))

        # ---- input DMAs (SP queue).  cfg["loads"]: list of block counts for
        # the x loads (independent of the compute chunking), with "E"/"B"
        # entries marking where the ewt / bb loads are interleaved. ----
        X = big.tile([128, BLK, DIN], F16)
        Ewt = big.tile([128, DOUT], BF16)
        bbcol = big.tile([128, 1], FP)
        pos = 0
        for item in cfg["loads"]:
            if item == "E":
                nc.sync.dma_start(out=Ewt, in_=ins["ewt"])
            elif item == "B":
                nc.sync.dma_start(out=bbcol,
                                  in_=ins["bb"].rearrange("(o u) -> o u", u=1))
            else:
                nc.sync.dma_start(out=X[:, pos:pos + item, :],
                                  in_=xd[:, pos:pos + item, :])
                pos += item
        assert pos == BLK

        ident16 = big.tile([128, 128], F16)
        make_identity(nc, ident16)
        rbx_col = big.tile([128, 1], FP)
        nc.gpsimd.memset(rbx_col, float(cfg["rbx"]))
        zcol = big.tile([128, 1], FP)
        nc.gpsimd.memset(zcol, 0.0)

        ET = big.tile([128, BLK, DIN], BF16)
        L = big.tile([128, BLK, DOUT], FP)
        Y = big.tile([128, BLK, DOUT], F16)

        # Exp and Ln both live in the natural_log_exp_and_others table
        # (set 6): one table load, no per-chunk switches.
        nc.scalar.add_instruction(mybir.InstLoadActFuncSet(
            name=nc.get_next_instruction_name(), ins=[], outs=[],
            act_func_set_id=6))

        # PE warmup during the DMA fill window: continuous dummy matmuls
        # ramp the p-state toward max clock before the first transpose.
        warm = psum.tile([128, GBMAX, 128], FP, tag="xt", bufs=2)
        for _ in range(WARM):
            nc.tensor.matmul(warm[:, 0, :], lhsT=ident16, rhs=ident16)

        XTs = [None] * NG
        Ss = [None] * NG

        def transp(j):
            cb, s = CS[j], ST_[j]
            XT = psum.tile([128, GBMAX, 128], FP, tag="xt", bufs=2)
            for b in range(cb):
                # matmul-with-identity == transpose: out[i, n] = X[n, i]
                nc.tensor.matmul(XT[:, b, :], lhsT=X[:, s + b, :], rhs=ident16)
            XTs[j] = XT

        def mm(j):
            cb, s = CS[j], ST_[j]
            S = psum.tile([128, GBMAX, 128], FP, tag="s", bufs=2)
            for b in range(cb):
                # output-transposed: S[o, n] = sum_i Ewt[i, o] ET[i, n]
                nc.tensor.matmul(S[:, b, :], lhsT=Ewt, rhs=ET[:, s + b, :])
            Ss[j] = S

        def exp(j):
            cb, s = CS[j], ST_[j]
            nc.scalar.activation(ET[:, s:s + cb, :], XTs[j][:, 0:cb, :],
                                 AF.Exp, bias=rbx_col, scale=-t)

        def ln(j):
            cb, s = CS[j], ST_[j]
            nc.scalar.activation(L[:, s:s + cb, :], Ss[j][:, 0:cb, :],
                                 AF.Ln, bias=zcol, scale=1.0)

        def fuse_store(j):
            cb, s = CS[j], ST_[j]
            nc.vector.tensor_scalar(out=Y[:, s:s + cb, :],
                                    in0=L[:, s:s + cb, :],
                                    scalar1=float(-1.0 / t), scalar2=bbcol,
                                    op0=OP.mult, op1=OP.add)
            nc.sync.dma_start(out=yd[:, s:s + cb, :], in_=Y[:, s:s + cb, :])

        # Software pipeline; per-engine streams stay in dependency-ready
        # order: PE: T0 T1 M0 T2 M1 ...; ACT: e0 e1 l0 e2 l1 ...
        for j in range(NG + 1):
            if j < NG:
                transp(j)
            if j >= 1:
                mm(j - 1)
            if j < NG:
                exp(j)
            if j >= 1:
                ln(j - 1)
                fuse_store(j - 1)


def build_nc(shard_rows: int, weight=None, *, t=16.0, rbx=-6.0,
             chunks=None, loads=None):
    nc = bacc.Bacc()
    x_d = nc.dram_tensor("x", [shard_rows, DIN], F16, kind="ExternalInput")
    ewt_d = nc.dram_tensor("ewt", [DIN, DOUT], BF16, kind="ExternalInput")
    bb_d = nc.dram_tensor("bb", [DOUT], FP, kind="ExternalInput")
    y_d = nc.dram_tensor("y", [DOUT, shard_rows], F16, kind="ExternalOutput")
    chunks = chunks or make_chunks(shard_rows // 128)
    if loads is None:
        loads = make_loads(chunks)
    cfg = dict(t=t, rbx=rbx, shard_rows=shard_rows, chunks=chunks,
               loads=loads)
    with tile.TileContext(nc) as tc:
        minplus_body(tc, {"y": y_d[:]},
                     {"x": x_d[:], "ewt": ewt_d[:], "bb": bb_d[:]}, cfg)
    nc.compile()
    return nc


def _host_prep(x2: np.ndarray, weight: np.ndarray, bias: np.ndarray):
    """Constants + the (tiny) weight-side operands, computed on host."""
    import ml_dtypes

    x16 = x2.astype(np.float16)
    x64 = x16.astype(np.float64)
    x_max = float(np.abs(x64).max())
    amin = float(x64.min())
    w2 = weight.astype(np.float64) + bias.astype(np.float64)[:, None]
    bo = w2.min(axis=1)
    rw = float(np.ptp(w2, axis=1).max())

    # Exact upper bound on q = min_i(x+w2) - b_o via the K smallest-w2
    # columns per output row (range calibration for the Ln domain).
    K = min(16, w2.shape[1])
    idx = np.argsort(w2, axis=1)[:, :K]                      # [o, K]
    wg = (np.take_along_axis(w2, idx, 1) - bo[:, None])[None].astype(np.float32)
    qmax = -np.inf
    xs = x16.astype(np.float32)
    for i in range(0, xs.shape[0], 4096):
        qK = (xs[i:i + 4096][:, idx] + wg).min(-1)
        qmax = max(qmax, float(qK.max()))

    t, rbx, rbw = softmin_cfg(x_max, rw, amin, qmax)

    ewt = np.exp(-t * (w2 - bo[:, None]) + rbw).T            # [i, o]
    ewt16 = np.ascontiguousarray(ewt.astype(np.float32)).astype(ml_dtypes.bfloat16)

    # CSHIFT: systematic softmin-vs-min gap, calibrated on a host sample.
    n = x2.shape[0]
    s = x64[:: max(1, n // 512)][:512]
    gmax = 0.0
    for i in range(0, s.shape[0], 128):
        v = s[i:i + 128, None, :] + w2[None, :, :]
        vmin = v.min(-1)
        sm = vmin - np.log(np.exp(-t * (v - vmin[..., None])).sum(-1)) / t
        gmax = max(gmax, float((vmin - sm).max()))
    cshift = 0.55 * gmax    # slight overshoot: global max gap exceeds sample's

    bb = (bo + (rbx + rbw) / t + cshift).astype(np.float32)
    return x16, ewt16, bb, t, rbx


def kernel(x: np.ndarray, weight: np.ndarray, bias: np.ndarray) -> np.ndarray:
    prefix = x.shape[:-1]
    x2 = np.ascontiguousarray(x, dtype=np.float32).reshape(-1, DIN)
    n = x2.shape[0]
    step = N_CORES * 128 * 4
    n_pad = (n + step - 1) // step * step
    if n_pad != n:
        x2 = np.concatenate([x2, np.zeros((n_pad - n, DIN), np.float32)], 0)
    shard = n_pad // N_CORES

    x16, ewt16, bb, t, rbx = _host_prep(x2, weight, bias)

    nc = build_nc(shard, t=t, rbx=rbx)
    in_maps = [{"x": np.ascontiguousarray(x16[c * shard:(c + 1) * shard]),
                "ewt": ewt16, "bb": bb} for c in range(N_CORES)]
    res = bass_utils.run_bass_kernel_spmd(nc, in_maps,
                                          core_ids=list(range(N_CORES)))
    blk = shard // 128
    parts = []
    for c in range(N_CORES):
        yt = np.asarray(res.results[c]["y"]).reshape(DOUT, blk, 128)
        # y_t[o, blk, p] -> y[p*blk + blk_idx, o]
        parts.append(np.transpose(yt, (2, 1, 0)).reshape(shard, DOUT))
    y = np.concatenate(parts, axis=0)
    return y[:n].astype(np.float32).reshape(*prefix, DOUT)


if __name__ == "__main__":
    rng = np.random.default_rng(0)
    x = rng.standard_normal((16, 2048, 128)).astype(np.float32)
    w = rng.standard_normal((128, 128)).astype(np.float32)
    b = rng.standard_normal(128).astype(np.float32)
    y = kernel(x, w, b)
    ref = (x[..., None, :] + w[None, None, :, :]).min(-1) + b
    err = np.abs(y - ref)
    print("max err:", err.max(), "rel absmax:", err.max() / np.abs(ref).max())
